# revision 7
# baseline (speedup 1.0000x reference)
"""Trainium2 Bass kernel for DifferentiableRankIntegration (quad-packed G=32).

Math (per query row i, B=1024):
  sig[k,j] = sigmoid((s[i,k] - s[i,j]) / tau),  tau = 0.1
  Sp[j] = sum_k pos[i,k]*sig[k,j],  Sn[j] = sum_k neg[i,k]*sig[k,j]
  rank[j] = 1 + Sn[j] if pos[i,j] else 1 + Sp[j]
  out[i,j] = (K+1) * (w_v/(K+rank_v) + w_l/(K+rank_l)),  K = 60

Grid factorization with G=32 points on [-5.2, 5.2]:
  pass-1: F[m] = sum_k m_k sigmoid(10(s_k - g_m))          (ACT + PE)
  filter: W = A @ F, A a dense [G,G] matrix fitted offline by two-sided
          ridge LSQ so that sum_mu (A F)_mu sigmoid((g_mu - x)/wrec)
          reproduces a single sigmoid step at any center (rel err of the
          full output ~7e-4, budget 2e-2)
  pass-2: f(s_j) ~= sum_mu W_mu sigmoid((g_mu - s_j)/wrec)

Layout: 4 rows ("quad") share each [128, 2048] ACT instruction.
  pass-1 E: [p = k mod 128, (q, c, t, m)], diff built by one DVE op with
          broadcast access patterns from resident sT10c.
  pass-2 phi: [p = (t, m), (q, j)], input broadcast by 4 GPSIMD
          partition_broadcast ops of [32, 2048] each; bias = g/wrec.
  pass-1 PE: per (q, c) one matmul lhsT=E-block [128, 128=(t,m)] x
          rhs=wint8 [128, 8=(t,mask)] -> Fps[(t,m), (t',mask)]; only the
          t==t' diagonal blocks are meaningful.
  filter PE: per (q, t) lhsT=TdSel_t (A.T in diagonal block t, zeros
          elsewhere) kills the off-diagonal garbage AND zero-pads F'.
  pass-2 PE: per (q, c) lhsT=phi-block [128, 128 j] x rhs=F' [128, 8]
          -> acc[j, (t,mask)]: j on partitions, so eviction is ONE
          [128, 128] DVE copy per quad into resident spc.
Finals are batched [128, 1024] VectorE ops in the j-partition layout
(pos/w inputs host-prepped into the same layout; host unshards).
"""

import numpy as np

B = 1024
NCORES = 8
ROWS = B // NCORES  # 128 rows per core
P = 128
NCHUNK = B // P  # 8
TAU = 0.1
K = 60.0

G = 32
NT = 4  # rows per quad
NQUAD = ROWS // NT  # 32
LO, HI = -5.2, 5.2
H = (HI - LO) / (G - 1)
WREC = 0.65 * H


def _sigmoid(x):
    return 1.0 / (1.0 + np.exp(-np.clip(x, -80, 80)))


def _fit_A(lam=1e-7, npts=1601):
    g = np.linspace(LO, HI, G)
    xs = np.linspace(LO - 0.5, HI + 0.5, npts)
    ss = xs
    Phi = _sigmoid((g[None, :] - xs[:, None]) / WREC)  # [X, G]
    Bm = _sigmoid((ss[None, :] - g[:, None]) / TAU)  # [G, S]
    T = _sigmoid((ss[None, :] - xs[:, None]) / TAU)  # [X, S]
    wx = np.exp(-0.25 * xs**2) + 1e-3
    ws = wx
    Phi_w = Phi * wx[:, None]
    T_w = T * wx[:, None] * ws[None, :]
    Bm_w = Bm * ws[None, :]
    PtP = Phi_w.T @ Phi_w + lam * np.trace(Phi_w.T @ Phi_w) / G * np.eye(G)
    BBt = Bm_w @ Bm_w.T + lam * np.trace(Bm_w @ Bm_w.T) / G * np.eye(G)
    A = np.linalg.solve(PtP, Phi_w.T @ T_w @ Bm_w.T) @ np.linalg.inv(BBt)
    return g, A


def _build_consts():
    g, A = _fit_A()
    grid10 = np.tile((10.0 * g)[None, :], (P, 1)).astype(np.float32)
    # TdSel_t [128, 128]: diagonal block t = A.T, zeros elsewhere.
    tdsel = np.zeros((P, NT * P), np.float64)
    for t in range(NT):
        tdsel[32 * t : 32 * t + 32, t * P + 32 * t : t * P + 32 * t + 32] = A.T
    biasg = (g[np.arange(P) % G] / WREC).reshape(P, 1)
    return (
        grid10,
        tdsel.astype(np.float32),
        biasg.astype(np.float32),
    )


GRID10, TDSEL, BIASG = _build_consts()


def _build_bass():
    import concourse.bacc as bacc
    import concourse.mybir as mybir
    from concourse.tile import TileContext

    f32 = mybir.dt.float32
    bf16 = mybir.dt.bfloat16

    nc = bacc.Bacc()

    # Per-core inputs (host pre-sharded / pre-transposed):
    # sT10c[p, ((q*8+c)*4+t)*32 + Q] = 10 * s_q[4Q+t, c*128+p]
    sT10c = nc.declare_dram_parameter("sT10c", [P, 2 * B], f32, isOutput=False)
    # bc2q[32t+m, Q*2048 + q*1024 + j] = s_q[4Q+t, j]  (pass-2 input, replicated)
    bc2q = nc.declare_dram_parameter("bc2q", [P, NQUAD * 2 * B], bf16, isOutput=False)
    # wint8[p, (Q*8+c)*8 + t*2 + mask] = mask[4Q+t, c*128+p] (0=pos, 1=neg)
    wint8 = nc.declare_dram_parameter("wint8", [P, 2 * B], bf16, isOutput=False)
    grid10 = nc.declare_dram_parameter("grid10", [P, G], f32, isOutput=False)
    tdsel = nc.declare_dram_parameter("tdsel", [P, NT * P], f32, isOutput=False)
    biasg = nc.declare_dram_parameter("biasg", [P, 1], f32, isOutput=False)
    # finals inputs in j-partition layout: [jp, (Q*8+c)*4 + t]
    posj = nc.declare_dram_parameter("posj", [P, B], f32, isOutput=False)
    wvj = nc.declare_dram_parameter("wvj", [P, B], f32, isOutput=False)
    wlj = nc.declare_dram_parameter("wlj", [P, B], f32, isOutput=False)
    out = nc.declare_dram_parameter("out", [P, B], f32, isOutput=True)

    with TileContext(nc) as tc:
        with (
            tc.tile_pool(name="const", bufs=1) as cpool,
            tc.tile_pool(name="diff", bufs=2) as dpool,
            tc.tile_pool(name="esig", bufs=2) as epool,
            tc.tile_pool(name="bcast", bufs=2) as bpool,
            tc.tile_pool(name="phi", bufs=2) as phpool,
            tc.tile_pool(name="fsb", bufs=2) as fbpool,
            tc.tile_pool(name="fin", bufs=1) as fpool,
            tc.tile_pool(name="psum_f", bufs=2, space="PSUM") as ppool_f,
            tc.tile_pool(name="psum_g", bufs=2, space="PSUM") as ppool_g,
            tc.tile_pool(name="psum_a", bufs=2, space="PSUM") as ppool_a,
        ):
            # --- load resident inputs ---
            sT_t = cpool.tile([P, 2 * B], f32, tag="sT")
            wint_t = cpool.tile([P, 2 * B], bf16, tag="wint")
            grid10_t = cpool.tile([P, G], f32, tag="grid10")
            tdsel_t = cpool.tile([P, NT * P], f32, tag="tdsel")
            biasg_t = cpool.tile([P, 1], f32, tag="biasg")
            nc.sync.dma_start(out=sT_t[:], in_=sT10c[:])
            nc.sync.dma_start(out=wint_t[:], in_=wint8[:])
            nc.sync.dma_start(out=grid10_t[:], in_=grid10[:])
            nc.sync.dma_start(out=tdsel_t[:], in_=tdsel[:])
            nc.sync.dma_start(out=biasg_t[:], in_=biasg[:])

            spc = fpool.tile([P, NQUAD * P], f32, tag="spc")

            # Sp/Sn destination tiles (filled row by row)
            for Q in range(NQUAD):
                # pass-1 diff: [p, (q, c, t, m)] = 10*s - 10*g via one DVE op
                diff_t = dpool.tile([P, 2 * B], bf16, tag="diff")
                scols = sT_t[:, Q : 2 * B : NQUAD].broadcast_to((P, 64, G))
                gbc = grid10_t[:, None, :].broadcast_to((P, 64, G))
                nc.vector.tensor_sub(
                    diff_t[:].rearrange("p (x m) -> p x m", x=64), scols, gbc
                )
                esig = epool.tile([P, 2 * B], bf16, tag="esig")
                nc.scalar.activation(
                    out=esig[:],
                    in_=diff_t[:],
                    func=mybir.ActivationFunctionType.Sigmoid,
                    bias=0.0,
                    scale=1.0,
                )
                # pass-2 input: bc2[(t, m), (q, j)] = s_q[4Q+t, j] (DMA'd replicated)
                bc2 = bpool.tile([P, 2 * B], bf16, tag="bc")
                nc.sync.dma_start(
                    out=bc2[:], in_=bc2q[:, Q * 2 * B : (Q + 1) * 2 * B]
                )
                phi = phpool.tile([P, 2 * B], bf16, tag="phi")
                nc.scalar.activation(
                    out=phi[:],
                    in_=bc2[:],
                    func=mybir.ActivationFunctionType.Sigmoid,
                    bias=biasg_t[:, 0:1],
                    scale=-1.0 / WREC,
                )
                # pass-1 contraction: Fps[(t,m), (q, t', mask)]
                fps = ppool_f.tile([P, 16], f32, tag="fps")
                for q in range(2):
                    for c in range(NCHUNK):
                        nc.tensor.matmul(
                            out=fps[:, 8 * q : 8 * q + 8],
                            lhsT=esig[:, (q * 8 + c) * P : (q * 8 + c + 1) * P],
                            rhs=wint_t[:, (Q * 8 + c) * 8 : (Q * 8 + c) * 8 + 8],
                            start=(c == 0),
                            stop=(c == NCHUNK - 1),
                        )
                fsb = fbpool.tile([P, 16], f32, tag="fsb")
                nc.vector.tensor_copy(fsb[:], fps[:])
                # filter: W = A @ F per (q, t), kills off-diagonal garbage
                f2ps = ppool_g.tile([P, 16], f32, tag="f2ps")
                for q in range(2):
                    for t in range(NT):
                        nc.tensor.matmul(
                            out=f2ps[:, 8 * q + 2 * t : 8 * q + 2 * t + 2],
                            lhsT=tdsel_t[:, t * P : (t + 1) * P],
                            rhs=fsb[:, 8 * q + 2 * t : 8 * q + 2 * t + 2],
                            start=True,
                            stop=True,
                        )
                f2sb = fbpool.tile([P, 16], bf16, tag="f2sb")
                nc.vector.tensor_copy(f2sb[:], f2ps[:])
                # pass-2: acc[j, (q, c, t, mask)]
                acc = ppool_a.tile([P, P], f32, tag="acc")
                for q in range(2):
                    for c in range(NCHUNK):
                        nc.tensor.matmul(
                            out=acc[:, (q * 8 + c) * 8 : (q * 8 + c) * 8 + 8],
                            lhsT=phi[:, q * B + c * P : q * B + (c + 1) * P],
                            rhs=f2sb[:, 8 * q : 8 * q + 8],
                            start=True,
                            stop=True,
                        )
                nc.vector.tensor_copy(spc[:, Q * P : (Q + 1) * P], acc[:])

            # --- finals, batched in j-partition layout ---
            pos_t = fpool.tile([P, B], f32, tag="pos")
            wv_t = fpool.tile([P, B], f32, tag="wv")
            wl_t = fpool.tile([P, B], f32, tag="wl")
            nc.sync.dma_start(out=pos_t[:], in_=posj[:])
            nc.sync.dma_start(out=wv_t[:], in_=wvj[:])
            nc.sync.dma_start(out=wl_t[:], in_=wlj[:])

            # spc free layout: (Q, q, c, t, mask)
            spc_r = spc[:].rearrange(
                "p (Q q c t m) -> p q m Q c t", Q=NQUAD, q=2, c=NCHUNK, t=NT, m=2
            )
            res = fpool.tile([P, B], f32, tag="res")
            t_v = fpool.tile([P, B], f32, tag="t_v")
            for q, (w_t, dst) in enumerate(((wv_t, None), (wl_t, res))):
                sp = spc_r[:, q, 0]
                sn = spc_r[:, q, 1]
                d1 = fpool.tile([P, B], f32, tag=f"d1_{q}")
                d1r = d1[:].rearrange("p (Q c t) -> p Q c t", Q=NQUAD, c=NCHUNK, t=NT)
                pos_r = pos_t[:].rearrange(
                    "p (Q c t) -> p Q c t", Q=NQUAD, c=NCHUNK, t=NT
                )
                nc.vector.tensor_sub(d1r, sn, sp)
                nc.vector.tensor_mul(d1r, pos_r, d1r)
                nc.vector.tensor_add(d1r, d1r, sp)
                # den = K + 1 + count = 61 + d1
                nc.vector.tensor_scalar_add(d1[:], d1[:], K + 1.0)
                nc.vector.reciprocal(d1[:], d1[:])
                if dst is None:
                    nc.vector.tensor_mul(t_v[:], w_t[:], d1[:])
                else:
                    nc.vector.tensor_mul(d1[:], w_t[:], d1[:])
                    nc.vector.tensor_add(res[:], t_v[:], d1[:])
            nc.sync.dma_start(out=out[:], in_=res[:])

    nc.compile()
    return nc


_NC_CACHE = None


def _get_nc():
    global _NC_CACHE
    if _NC_CACHE is None:
        _NC_CACHE = _build_bass()
    return _NC_CACHE


def _prep_core_inputs(s_v, s_l, pos_f, neg_f, w_v, w_l, core):
    import ml_dtypes

    lo, hi = core * ROWS, (core + 1) * ROWS
    sv = np.ascontiguousarray(s_v[lo:hi]).astype(np.float32)
    sl = np.ascontiguousarray(s_l[lo:hi]).astype(np.float32)

    # sT10c[p, q, c, t, Q] = 10*s_q[4Q+t, c*128+p]
    arr = np.stack([sv, sl])  # [q, r, j]
    a5 = arr.reshape(2, NQUAD, NT, NCHUNK, P)  # [q, Q, t, c, p]
    sT10c = 10.0 * a5.transpose(4, 0, 3, 2, 1).reshape(P, 2 * B)

    # bc2q[32t+m, Q, q, j] = s_q[4Q+t, j]
    aq = arr.reshape(2, NQUAD, NT, B).transpose(2, 1, 0, 3)  # [t, Q, q, j]
    bc2q = np.broadcast_to(
        aq[:, None, :, :, :], (NT, G, NQUAD, 2, B)
    ).reshape(P, NQUAD * 2 * B)

    # wint8[p, Q, c, t, mask]
    masks = np.stack([pos_f[lo:hi], neg_f[lo:hi]])  # [mask, r, j]
    m5 = masks.reshape(2, NQUAD, NT, NCHUNK, P)  # [mask, Q, t, c, p]
    wint8 = m5.transpose(4, 1, 3, 2, 0).reshape(P, 2 * B)

    def jlay(x):
        # [jp, (Q*8+c)*4 + t] = x[4Q+t, c*128+jp]
        x4 = np.asarray(x, np.float32).reshape(NQUAD, NT, NCHUNK, P)
        return np.ascontiguousarray(x4.transpose(3, 0, 2, 1).reshape(P, B))

    return {
        "sT10c": np.ascontiguousarray(sT10c.astype(np.float32)),
        "bc2q": np.ascontiguousarray(bc2q).astype(ml_dtypes.bfloat16),
        "wint8": np.ascontiguousarray(wint8).astype(ml_dtypes.bfloat16),
        "grid10": GRID10,
        "tdsel": TDSEL,
        "biasg": BIASG,
        "posj": jlay(pos_f[lo:hi]),
        "wvj": jlay((K + 1.0) * w_v[lo:hi]),
        "wlj": jlay((K + 1.0) * w_l[lo:hi]),
    }


def _unshard_core_out(o):
    # o[jp, (Q*8+c)*4 + t] -> [r = 4Q+t, j = c*128+jp]
    o4 = o.reshape(P, NQUAD, NCHUNK, NT)
    return o4.transpose(1, 3, 2, 0).reshape(ROWS, B)


def _run(in_maps, trace=False):
    from concourse.bass_utils import run_bass_kernel_spmd

    nc = _get_nc()
    return run_bass_kernel_spmd(nc, in_maps, core_ids=list(range(NCORES)), trace=trace)


def kernel(s_v, s_l, pos_mask, neg_mask, w_v, w_l, _trace=False):
    pos_f = pos_mask.astype(np.float32)
    neg_f = neg_mask.astype(np.float32)
    in_maps = [
        _prep_core_inputs(s_v, s_l, pos_f, neg_f, w_v, w_l, core)
        for core in range(NCORES)
    ]
    res = _run(in_maps, trace=_trace)
    outs = [
        _unshard_core_out(np.asarray(res.results[i]["out"], np.float32))
        for i in range(NCORES)
    ]
    full = np.concatenate(outs, axis=0).astype(np.float32)
    if _trace:
        return full, res
    return full
